# revision 1
# baseline (speedup 1.0000x reference)
"""TRN2 Bass kernel for nn_CustomBlock (cosine-normalized channel attention).

Per group n (8 groups -> 8 NeuronCores, pure data parallel):
  K = Wk @ X + Wk0;  Q = Wq @ X + Wq0            (X: [C,B])
  S[i,j] = sum_b Q[i,b] K[j,b]
  Y = S / sqrt(max(|Q_i|^2,eps') * max(|K_j|^2,eps'))
  SM = softmax over i (per column j); Z[j,b] = sum_i SM[i,j] X[i,b]

Implementation (single core, all matmuls float32r = full-rate fp32):
  phase 1: KT[b,j], QT[b,i] via matmuls with X as lhsT (transposed-producing);
           KT resident in SBUF (4 pieces), QT spilled to DRAM; row/col norms
           accumulated on the fly.
  phase 2: S row-panels = QT_panel^T @ KT; E = exp(rq_i * (S * rk_j)) -> DRAM;
           column sums of E in persistent PSUM accumulators (ones-matmul).
  phase 3: Z row-panels = E_colpanel^T @ X (X re-loaded into KT's SBUF slots),
           scaled by 1/colsum per partition.
"""

import os
import sys
import time

import numpy as np

try:
    import concourse.bass as bass  # noqa: F401
except ImportError:
    for _p in (
        "/opt/trn_rl_repo",
        "/opt/pypackages",
        "/root/.axon_site/_ro/trn_rl_repo",
        "/root/.axon_site/_ro/pypackages",
    ):
        if _p not in sys.path:
            sys.path.append(_p)

import jax
import concourse.bacc as bacc
import concourse.mybir as mybir
import concourse.tile as tile

P = 128
F32 = mybir.dt.float32
F32R = mybir.dt.float32r
AF = mybir.ActivationFunctionType
OP = mybir.AluOpType

N_CORES = 8
FULL_C = 2048
FULL_B = 2048

LAST_EXEC_NS = None


def build_program(C, B):
    """Build the single-core Bass program (same program for all cores)."""
    nc = bacc.Bacc("TRN2", target_bir_lowering=False, debug=False,
                   num_devices=N_CORES)

    CT = C // P           # channel tiles (i/j/c)
    BT = B // P           # b tiles
    SL = 256              # phase-1 output slice width
    NSL = C // SL
    S2 = 512
    NJ2 = C // S2         # phase-2 j slices
    NB3 = B // S2         # phase-3 b slices
    NPIECE = 4 if BT % 4 == 0 else 1
    PIECE = BT // NPIECE  # b-tiles per resident piece

    x_d = nc.dram_tensor("x", [C, B], F32R, kind="ExternalInput").ap()
    wkt_d = nc.dram_tensor("wkt", [C + P, C], F32R, kind="ExternalInput").ap()
    wqt_d = nc.dram_tensor("wqt", [C + P, C], F32R, kind="ExternalInput").ap()
    z_d = nc.dram_tensor("z", [C, B], F32, kind="ExternalOutput").ap()

    with tile.TileContext(nc) as tc:
        with (
            tc.tile_pool(name="dram", bufs=1, space="DRAM") as dram,
            tc.tile_pool(name="big0", bufs=1) as big0,
            tc.tile_pool(name="big1", bufs=1) as big1,
            tc.tile_pool(name="big2", bufs=1) as big2,
            tc.tile_pool(name="big3", bufs=1) as big3,
            tc.tile_pool(name="w", bufs=2) as wpool,
            tc.tile_pool(name="str", bufs=2) as spool,
            tc.tile_pool(name="stage", bufs=2) as stpool,
            tc.tile_pool(name="tmp", bufs=2) as tmppool,
            tc.tile_pool(name="zrow", bufs=2) as zpool,
            tc.tile_pool(name="row", bufs=2) as rowpool,
            tc.tile_pool(name="stat", bufs=1) as stat,
            tc.tile_pool(name="ps", bufs=4, space="PSUM") as ps,
            tc.tile_pool(name="pscol", bufs=4, space="PSUM") as pscol,
        ):
            bigs = [big0, big1, big2, big3][:NPIECE]
            qt_dm = dram.tile([B, C], F32R, tag="qt")
            e_dm = dram.tile([C, C], F32R, tag="e")
            rk_dm = dram.tile([1, C], F32R, tag="rk")
            col_dm = dram.tile([C], F32, tag="col")

            ones_row = stat.tile([1, P], F32R, tag="ones_row")
            ones_colr = stat.tile([P, 1], F32R, tag="ones_colr")
            ones_col = stat.tile([P, 1], F32, tag="ones_col")
            rq = stat.tile([P, CT], F32, tag="rq")
            rcol = stat.tile([P, CT], F32, tag="rcol")
            ones_row_f = stat.tile([1, P], F32, tag="ones_row_f")
            nc.vector.memset(ones_row_f[:], 1.0)
            nc.vector.memset(ones_col[:], 1.0)
            nc.scalar.copy(ones_row[:], ones_row_f[:])
            nc.scalar.copy(ones_colr[:], ones_col[:])

            kt = [
                bigs[i].tile([P, PIECE, C], F32R, tag=f"big{i}",
                             name=f"kt{i}")
                for i in range(NPIECE)
            ]

            x_r = x_d.rearrange("(ct p) b -> p ct b", p=P)
            wkt_r = wkt_d.rearrange("(ct p) j -> p ct j", p=P)
            wqt_r = wqt_d.rearrange("(ct p) j -> p ct j", p=P)

            # ---------------- phase 1: K/Q projections ----------------
            for sl in range(NSL):
                js = slice(sl * SL, (sl + 1) * SL)
                wkp = wpool.tile([P, CT + 1, SL], F32R, tag="w")
                nc.sync.dma_start(wkp[:], wkt_r[:, :, js])
                wqp = wpool.tile([P, CT + 1, SL], F32R, tag="w")
                nc.sync.dma_start(wqp[:], wqt_r[:, :, js])
                ssk = stat.tile([P, SL], F32, tag="ssk")
                ssq = stat.tile([P, SL], F32, tag="ssq")
                for bt in range(BT):
                    xcol = spool.tile([P, CT, P], F32R, tag="str")
                    nc.sync.dma_start(
                        xcol[:], x_r[:, :, bt * P : (bt + 1) * P]
                    )
                    # K tile [128b, SL j]
                    psk = ps.tile([P, SL], F32, tag="ps")
                    for ct in range(CT):
                        nc.tensor.matmul(psk[:], xcol[:, ct, :],
                                         wkp[:, ct, :],
                                         start=(ct == 0), stop=False)
                    nc.tensor.matmul(psk[:], ones_row[:],
                                     wkp[0:1, CT, :],
                                     start=False, stop=True)
                    nc.scalar.copy(
                        kt[bt // PIECE][:, bt % PIECE, js], psk[:]
                    )
                    if bt == 0:
                        nc.scalar.square(ssk[:], psk[:])
                    else:
                        sq = tmppool.tile([P, SL], F32, tag="tmp")
                        nc.scalar.square(sq[:], psk[:])
                        nc.vector.tensor_tensor(ssk[:], ssk[:], sq[:],
                                                OP.add)
                    # Q tile
                    psq = ps.tile([P, SL], F32, tag="ps")
                    for ct in range(CT):
                        nc.tensor.matmul(psq[:], xcol[:, ct, :],
                                         wqp[:, ct, :],
                                         start=(ct == 0), stop=False)
                    nc.tensor.matmul(psq[:], ones_row[:],
                                     wqp[0:1, CT, :],
                                     start=False, stop=True)
                    qst = stpool.tile([P, SL], F32R, tag="stage")
                    nc.scalar.copy(qst[:], psq[:])
                    nc.sync.dma_start(
                        qt_dm[bt * P : (bt + 1) * P, js], qst[:]
                    )
                    if bt == 0:
                        nc.scalar.square(ssq[:], psq[:])
                    else:
                        sq2 = tmppool.tile([P, SL], F32, tag="tmp")
                        nc.scalar.square(sq2[:], psq[:])
                        nc.vector.tensor_tensor(ssq[:], ssq[:], sq2[:],
                                                OP.add)
                # DK2 row for this slice -> rk
                pr = pscol.tile([1, SL], F32, tag="pscol")
                nc.tensor.matmul(pr[:], ones_col[:], ssk[:],
                                 start=True, stop=True)
                r1 = zpool.tile([1, SL], F32, tag="zrow")
                nc.vector.tensor_scalar(r1[:], pr[:], 1e-6, None, OP.max)
                r2 = zpool.tile([1, SL], F32, tag="zrow")
                nc.scalar.sqrt(r2[:], r1[:])
                r3 = rowpool.tile([1, SL], F32R, tag="row")
                with nc.allow_low_precision(
                    reason="rk stored f32r for matmul broadcast"
                ):
                    nc.vector.reciprocal(r3[:], r2[:])
                nc.sync.dma_start(rk_dm[0:1, js], r3[:])
                # DQ2 columns for this slice -> rq
                for k in range(SL // P):
                    pq = pscol.tile([P, 1], F32, tag="pscol")
                    nc.tensor.matmul(pq[:],
                                     ssq[:, k * P : (k + 1) * P],
                                     ones_col[:], start=True, stop=True)
                    c1 = tmppool.tile([P, 1], F32, tag="tmp")
                    nc.vector.tensor_scalar(c1[:], pq[:], 1e-6, None,
                                            OP.max)
                    c2 = tmppool.tile([P, 1], F32, tag="tmp")
                    nc.scalar.sqrt(c2[:], c1[:])
                    idx = sl * (SL // P) + k
                    nc.vector.reciprocal(rq[:, idx : idx + 1], c2[:])

            # ---------------- phase 2: scores, exp, colsum ----------------
            qt_r = qt_dm.rearrange("(bt p) i -> p bt i", p=P)
            RK = wpool.tile([P, NJ2, S2], F32, tag="w")
            for jsl in range(NJ2):
                rkrow = rowpool.tile([1, S2], F32R, tag="row")
                nc.sync.dma_start(
                    rkrow[:], rk_dm[0:1, jsl * S2 : (jsl + 1) * S2]
                )
                psb = ps.tile([P, S2], F32, tag="ps")
                nc.tensor.matmul(psb[:], ones_row[:], rkrow[:],
                                 start=True, stop=True)
                nc.scalar.copy(RK[:, jsl, :], psb[:])
            cs = [
                pscol.tile([1, S2], F32, tag="pscol", name=f"cs{j}")
                for j in range(NJ2)
            ]
            for ip in range(CT):
                qtp = spool.tile([P, BT, P], F32R, tag="str")
                nc.sync.dma_start(
                    qtp[:], qt_r[:, :, ip * P : (ip + 1) * P]
                )
                for jsl in range(NJ2):
                    js2 = slice(jsl * S2, (jsl + 1) * S2)
                    pss = ps.tile([P, S2], F32, tag="ps")
                    for bt in range(BT):
                        nc.tensor.matmul(
                            pss[:], qtp[:, bt, :],
                            kt[bt // PIECE][:, bt % PIECE, js2],
                            start=(bt == 0), stop=(bt == BT - 1),
                        )
                    tm = tmppool.tile([P, S2], F32, tag="tmp")
                    nc.vector.tensor_tensor(tm[:], pss[:], RK[:, jsl, :],
                                            OP.mult)
                    et = stpool.tile([P, S2], F32R, tag="stage")
                    nc.scalar.activation(et[:], tm[:], AF.Exp,
                                         scale=rq[:, ip : ip + 1])
                    nc.tensor.matmul(cs[jsl][:], ones_colr[:], et[:],
                                     start=(ip == 0), stop=(ip == CT - 1))
                    nc.sync.dma_start(
                        e_dm[ip * P : (ip + 1) * P, js2], et[:]
                    )

            # colsum -> rcol
            for jsl in range(NJ2):
                crow = zpool.tile([1, S2], F32, tag="zrow")
                nc.scalar.copy(crow[:], cs[jsl][:])
                nc.sync.dma_start(
                    col_dm[jsl * S2 : (jsl + 1) * S2].rearrange(
                        "(a c) -> a c", a=1
                    ),
                    crow[:],
                )
            rcr = zpool.tile([P, CT], F32, tag="zrow")
            nc.sync.dma_start(rcr[:], col_dm.rearrange("(o p) -> p o", p=P))
            nc.vector.reciprocal(rcol[:], rcr[:])

            # ---------------- phase 3: Z = SM^T X ----------------
            xp = []
            for pc in range(NPIECE):
                t = bigs[pc].tile([P, PIECE, B], F32R, tag=f"big{pc}",
                                  name=f"xp{pc}")
                for c in range(PIECE):
                    nc.sync.dma_start(
                        t[:, c, :], x_r[:, pc * PIECE + c, :]
                    )
                xp.append(t)
            e_r = e_dm.rearrange("(ic p) j -> p ic j", p=P)
            for jt in range(CT):
                ept = spool.tile([P, CT, P], F32R, tag="str")
                nc.sync.dma_start(
                    ept[:], e_r[:, :, jt * P : (jt + 1) * P]
                )
                for bsl in range(NB3):
                    bs2 = slice(bsl * S2, (bsl + 1) * S2)
                    psz = ps.tile([P, S2], F32, tag="ps")
                    for ic in range(CT):
                        nc.tensor.matmul(
                            psz[:], ept[:, ic, :],
                            xp[ic // PIECE][:, ic % PIECE, bs2],
                            start=(ic == 0), stop=(ic == CT - 1),
                        )
                    zt = zpool.tile([P, S2], F32, tag="zrow")
                    nc.scalar.mul(zt[:], psz[:], rcol[:, jt : jt + 1])
                    nc.sync.dma_start(
                        z_d[jt * P : (jt + 1) * P, bs2], zt[:]
                    )

    nc.compile()
    return nc


def _host_prep(Wk, Wq, Wk0, Wq0, C):
    wkt = np.concatenate(
        [np.ascontiguousarray(Wk.T),
         Wk0.reshape(1, C),
         np.zeros((P - 1, C), np.float32)], axis=0
    ).astype(np.float32)
    wqt = np.concatenate(
        [np.ascontiguousarray(Wq.T),
         Wq0.reshape(1, C),
         np.zeros((P - 1, C), np.float32)], axis=0
    ).astype(np.float32)
    return wkt, wqt


_CACHE = {}


def kernel(X, Wk, Wq, Wk0, Wq0):
    global LAST_EXEC_NS
    X = np.asarray(X, dtype=np.float32)
    Wk = np.asarray(Wk, dtype=np.float32)
    Wq = np.asarray(Wq, dtype=np.float32)
    Wk0 = np.asarray(Wk0, dtype=np.float32)
    Wq0 = np.asarray(Wq0, dtype=np.float32)
    N, C, B = X.shape
    assert N == N_CORES

    from concourse.bass_utils import run_bass_kernel_spmd

    key = (C, B)
    if key not in _CACHE:
        _CACHE[key] = build_program(C, B)
    nc = _CACHE[key]

    wkt, wqt = _host_prep(Wk, Wq, Wk0, Wq0, C)
    in_maps = [
        {"x": np.ascontiguousarray(X[n]), "wkt": wkt, "wqt": wqt}
        for n in range(N)
    ]
    t0 = time.time()
    res = run_bass_kernel_spmd(nc, in_maps, core_ids=list(range(N_CORES)))
    LAST_EXEC_NS = int((time.time() - t0) * 1e9)
    out = np.stack([res.results[n]["z"] for n in range(N)], axis=0)
    return out.astype(np.float32)


if __name__ == "__main__":
    # small-scale self-test vs numpy
    C, B = 512, 512
    rng = np.random.default_rng(1)
    Xs = rng.standard_normal((N_CORES, C, B), dtype=np.float32)
    bound = float(np.sqrt(6.0 / (C + C)))
    Wks = rng.uniform(-bound, bound, (C, C)).astype(np.float32)
    Wqs = rng.uniform(-bound, bound, (C, C)).astype(np.float32)
    Wk0s = rng.standard_normal((C, 1)).astype(np.float32) * 0.01
    Wq0s = rng.standard_normal((C, 1)).astype(np.float32) * 0.01

    def ref(X, Wk, Wq, Wk0, Wq0):
        K = np.einsum("ij,njb->nib", Wk, X) + Wk0
        Q = np.einsum("ij,njb->nib", Wq, X) + Wq0
        DK2 = np.sum(K * K, axis=2)
        DQ2 = np.sum(Q * Q, axis=2)
        DQK = np.sqrt(np.maximum(DQ2[:, :, None] * DK2[:, None, :], 1e-12))
        Y = np.einsum("nib,njb->nij", Q, K) / DQK
        Y = Y - Y.max(axis=1, keepdims=True)
        E = np.exp(Y)
        SM = E / E.sum(axis=1, keepdims=True)
        return np.einsum("ncb,ncj->njb", X, SM)

    expected = ref(
        Xs.astype(np.float64), Wks.astype(np.float64),
        Wqs.astype(np.float64), Wk0s.astype(np.float64),
        Wq0s.astype(np.float64),
    )
    actual = kernel(Xs, Wks, Wqs, Wk0s, Wq0s)
    rel = np.linalg.norm(actual - expected) / np.linalg.norm(expected)
    print(f"small test relative error: {rel:.3e}")
    print(f"wall ns: {LAST_EXEC_NS}")



# revision 20
# speedup vs baseline: 2.0245x; 2.0245x over previous
"""TRN2 Bass kernel for nn_CustomBlock (cosine-normalized channel attention).

Per group n (8 groups -> 8 NeuronCores, pure data parallel):
  K = Wk @ X + Wk0;  Q = Wq @ X + Wq0            (X: [C,B])
  S[i,j] = sum_b Q[i,b] K[j,b]
  cos = S / sqrt(max(|Q_i|^2,eps) * max(|K_j|^2,eps))
  SM = softmax over i (per column j); Z[j,b] = sum_i SM[i,j] X[i,b]

Implementation (single core):
  phase 1: KT[b,j], QT[b,i] via fp8 DoubleRow matmuls (X, W pre-quantized
           e4m3 on host; W scaled by 256, bias folded in via an indicator
           k-pair). QT kept resident in SBUF (fp8, scale 16); KT spilled
           to DRAM (fp8). Row norms from the fp32 PSUM accumulators.
  phase 2: per 256-wide j-panel: S-tiles = QT^T KT (fp8 DoubleRow),
           E = exp(cos) in bf16 kept in SBUF; column sums via ones-matmul.
  phase 3: Z panel = E^T X (bf16, X resident in SBUF), scaled by 1/colsum.
  Phases 2 and 3 are software-pipelined across panels.

fp8 error analysis: scores are cosines (|cos| <~ 0.15 for this data);
quantization noise enters as ~eps/sqrt(B) absolute in cos => ~1e-3,
negligible after exp. Phase 3 stays bf16 (fp8 there would put ~4% on Z).
"""

import os
import sys
import time

import numpy as np

try:
    import concourse.bass as bass  # noqa: F401
except ImportError:
    for _p in (
        "/opt/trn_rl_repo",
        "/opt/pypackages",
        "/root/.axon_site/_ro/trn_rl_repo",
        "/root/.axon_site/_ro/pypackages",
    ):
        if _p not in sys.path:
            sys.path.append(_p)

import ml_dtypes
import concourse.bacc as bacc
import concourse.mybir as mybir
import concourse.tile as tile

P = 128
F32 = mybir.dt.float32
BF16 = mybir.dt.bfloat16
FP8 = mybir.dt.float8e4
AF = mybir.ActivationFunctionType
OP = mybir.AluOpType
DR = mybir.MatmulPerfMode.DoubleRow

FP8NP = ml_dtypes.float8_e4m3
BF16NP = ml_dtypes.bfloat16

N_CORES = 8
FULL_C = 2048
FULL_B = 2048

# fp8 scales: W stored as 256*W, K/Q stored as 16*K (PSUM/16).
# Norms are taken on the fp8-rounded 16K values: sum(kst^2) = 256*DK2,
# so rk = rsqrt(max(sum kst^2, 256*eps)) = 1/(16*sqrt(max(DK2,eps)))
# and cos = S_psum * rk * rq exactly (S_psum = 256*S).
WSCALE = 256.0
KDIV = 1.0 / 16.0
EPS_SS = 1e-6 * WSCALE  # eps floor in sum(kst^2) units

LAST_EXEC_NS = None


def build_program(C, B):
    nc = bacc.Bacc("TRN2", target_bir_lowering=False, debug=False,
                   num_devices=N_CORES)

    CT = C // P           # channel tiles
    BT = B // P           # b tiles
    XT = CT + 2           # x8 tiles incl. bias indicator pair
    SL1 = min(512, C)     # phase-1 output slice width (i/j channels)
    NSL1 = C // SL1
    JP = 256              # phase-2/3 j-panel width
    NJP = C // JP
    BS3 = 512             # phase-3 b slice width
    NB3 = B // BS3
    XCH = max(1, CT // NSL1)  # xbf tiles DMA'd per phase-1 slice

    x8_d = nc.dram_tensor("x8", [C + 2 * P, B], FP8,
                          kind="ExternalInput").ap()
    xbf_d = nc.dram_tensor("xbf", [C, B], BF16, kind="ExternalInput").ap()
    wk8_d = nc.dram_tensor("wk8", [C + 2 * P, C], FP8,
                           kind="ExternalInput").ap()
    wq8_d = nc.dram_tensor("wq8", [C + 2 * P, C], FP8,
                           kind="ExternalInput").ap()
    z_d = nc.dram_tensor("z", [C, B], F32, kind="ExternalOutput").ap()

    from contextlib import ExitStack

    with tile.TileContext(nc) as tc, ExitStack() as stack:
        en = stack.enter_context
        dram = en(tc.tile_pool(name="dram", bufs=1, space="DRAM"))
        x8p = en(tc.tile_pool(name="x8p", bufs=1))
        xbfp = en(tc.tile_pool(name="xbfp", bufs=1))
        qtp = en(tc.tile_pool(name="qtp", bufs=1))
        wp = en(tc.tile_pool(name="wp", bufs=3))
        ktp = en(tc.tile_pool(name="ktp", bufs=2))
        ep = en(tc.tile_pool(name="ep", bufs=2))
        rkp = en(tc.tile_pool(name="rkp", bufs=2))
        sspool = en(tc.tile_pool(name="ss", bufs=2))
        stpool = en(tc.tile_pool(name="stage", bufs=2))
        zpool = en(tc.tile_pool(name="zp", bufs=2))
        tmppool = en(tc.tile_pool(name="tmp", bufs=2))
        smpool = en(tc.tile_pool(name="sm", bufs=2))
        csrpool = en(tc.tile_pool(name="csr", bufs=2))
        rcpool = en(tc.tile_pool(name="rcp", bufs=2))
        stat = en(tc.tile_pool(name="stat", bufs=1))
        ps = en(tc.tile_pool(name="ps", bufs=4, space="PSUM"))
        pscs = en(tc.tile_pool(name="pscs", bufs=1, space="PSUM"))
        psm = en(tc.tile_pool(name="psm", bufs=2, space="PSUM"))
        en(nc.allow_low_precision(
            reason="bf16 norm accumulators / fp8 staging; error bounded by "
                   "cosine normalization analysis in module docstring"))
        if True:
            kt_dm = dram.tile([B, C], FP8, tag="kt")

            ones_col = stat.tile([P, 1], F32, tag="ones_col")
            ones_row = stat.tile([1, P], F32, tag="ones_row")
            ones1 = stat.tile([1, 1], F32, tag="ones1")
            ones_colb = stat.tile([P, 1], BF16, tag="ones_colb")
            ones_rowb = stat.tile([1, P], BF16, tag="ones_rowb")
            rq = stat.tile([P, CT], F32, tag="rq")
            rk_all = stat.tile([1, C], BF16, tag="rk_all")
            nc.vector.memset(ones_col[:], 1.0)
            nc.vector.memset(ones_row[:], 1.0)
            nc.vector.memset(ones1[:], 1.0)
            nc.scalar.copy(ones_colb[:], ones_col[:])
            nc.scalar.copy(ones_rowb[:], ones_row[:])

            x8t = x8p.tile([P, XT, B], FP8, tag="x8")
            xbft = xbfp.tile([P, CT, B], BF16, tag="xbf")
            qt8 = qtp.tile([P, BT, C], FP8, tag="qt")

            x8_r = x8_d.rearrange("(t p) b -> p t b", p=P)
            xbf_r = xbf_d.rearrange("(t p) b -> p t b", p=P)
            wk8_r = wk8_d.rearrange("(t p) j -> p t j", p=P)
            wq8_r = wq8_d.rearrange("(t p) j -> p t j", p=P)
            kt_r = kt_dm.rearrange("(bt p) j -> p bt j", p=P)

            # chunked so the first DR pair's inputs land early
            for t in range(0, XT, 2):
                nc.sync.dma_start(x8t[:, t : t + 2, :],
                                  x8_r[:, t : t + 2, :])

            def load_w(src_r, js):
                w = wp.tile([P, XT, SL1], FP8, tag="w")
                step = max(2, XT // 3)
                for t in range(0, XT, step):
                    t1 = min(XT, t + step)
                    nc.sync.dma_start(w[:, t:t1, :], src_r[:, t:t1, js])
                return w

            # ---------------- phase 1: K/Q projections (fp8 DR) ----------
            def issue_norms(ssk, ssq, sl):
                js = slice(sl * SL1, (sl + 1) * SL1)
                pr = psm.tile([1, SL1], F32, tag="m")
                nc.tensor.matmul(pr[:], ones_colb[:], ssk[:],
                                 start=True, stop=True)
                r1 = smpool.tile([1, SL1], F32, tag="smr")
                nc.vector.tensor_scalar(r1[:], pr[:], EPS_SS, None,
                                        OP.max)
                r2 = smpool.tile([1, SL1], F32, tag="smr")
                nc.scalar.sqrt(r2[:], r1[:])
                nc.vector.reciprocal(rk_all[0:1, js], r2[:])
                prq = psm.tile([1, SL1], F32, tag="m")
                nc.tensor.matmul(prq[:], ones_colb[:], ssq[:],
                                 start=True, stop=True)
                q1 = smpool.tile([1, SL1], F32, tag="smr")
                nc.vector.tensor_scalar(q1[:], prq[:], EPS_SS, None,
                                        OP.max)
                q2 = smpool.tile([1, SL1], F32, tag="smr")
                nc.scalar.sqrt(q2[:], q1[:])
                for k in range(SL1 // P):
                    pq = psm.tile([P, 1], F32, tag="mc", bufs=1)
                    nc.tensor.matmul(pq[:], q2[0:1, k * P : (k + 1) * P],
                                     ones1[:], start=True, stop=True)
                    idx = sl * (SL1 // P) + k
                    nc.vector.reciprocal(rq[:, idx : idx + 1], pq[:])

            pending_norms = None
            for sl in range(NSL1):
                js = slice(sl * SL1, (sl + 1) * SL1)
                wk = load_w(wk8_r, js)
                wq = load_w(wq8_r, js)
                # trickle in the bf16 X copy (used only in phase 3)
                c0 = sl * XCH
                if c0 < CT:
                    c1 = min(CT, c0 + XCH)
                    nc.sync.dma_start(xbft[:, c0:c1, :], xbf_r[:, c0:c1, :])
                ssk = sspool.tile([P, SL1], BF16, tag="ssk")
                ssq = sspool.tile([P, SL1], BF16, tag="ssq")
                for bt in range(BT):
                    bs = slice(bt * P, (bt + 1) * P)
                    psk = ps.tile([P, SL1], F32, tag="ps")
                    for t in range(XT // 2):
                        nc.tensor.matmul(
                            psk[:], x8t[:, 2 * t : 2 * t + 2, bs],
                            wk[:, 2 * t : 2 * t + 2, :],
                            start=(t == 0), stop=(t == XT // 2 - 1),
                            perf_mode=DR,
                        )
                    kst = stpool.tile([P, SL1], FP8, tag="stage")
                    nc.scalar.mul(kst[:], psk[:], KDIV)
                    nc.sync.dma_start(kt_r[:, bt, js], kst[:])
                    if bt == 0:
                        nc.vector.tensor_tensor(ssk[:], kst[:], kst[:],
                                                OP.mult)
                    else:
                        sq = tmppool.tile([P, SL1], BF16, tag="tmp")
                        nc.vector.tensor_tensor(sq[:], kst[:], kst[:],
                                                OP.mult)
                        nc.vector.tensor_tensor(ssk[:], ssk[:], sq[:],
                                                OP.add)
                    psq = ps.tile([P, SL1], F32, tag="ps")
                    for t in range(XT // 2):
                        nc.tensor.matmul(
                            psq[:], x8t[:, 2 * t : 2 * t + 2, bs],
                            wq[:, 2 * t : 2 * t + 2, :],
                            start=(t == 0), stop=(t == XT // 2 - 1),
                            perf_mode=DR,
                        )
                    nc.scalar.mul(qt8[:, bt, js], psq[:], KDIV)
                    qs = qt8[:, bt, js]
                    if bt == 0:
                        nc.vector.tensor_tensor(ssq[:], qs, qs, OP.mult)
                    else:
                        sq2 = tmppool.tile([P, SL1], BF16, tag="tmp")
                        nc.vector.tensor_tensor(sq2[:], qs, qs, OP.mult)
                        nc.vector.tensor_tensor(ssq[:], ssq[:], sq2[:],
                                                OP.add)
                # rk slice: 16/sqrt(max(DK2psum, eps'))
                pr = psm.tile([1, SL1], F32, tag="m")
                nc.tensor.matmul(pr[:], ones_colb[:], ssk[:],
                                 start=True, stop=True)
                r1 = smpool.tile([1, SL1], F32, tag="smr")
                nc.vector.tensor_scalar(r1[:], pr[:], EPS_SS, None,
                                        OP.max)
                r2 = smpool.tile([1, SL1], F32, tag="smr")
                nc.scalar.sqrt(r2[:], r1[:])
                nc.vector.reciprocal(rk_all[0:1, js], r2[:])
                # rq: 16/sqrt(max(DQ2psum, eps')): row-reduce, then
                # transpose 128-chunks into partitions via tiny matmuls
                prq = psm.tile([1, SL1], F32, tag="m")
                nc.tensor.matmul(prq[:], ones_colb[:], ssq[:],
                                 start=True, stop=True)
                q1 = smpool.tile([1, SL1], F32, tag="smr")
                nc.vector.tensor_scalar(q1[:], prq[:], EPS_SS, None,
                                        OP.max)
                q2 = smpool.tile([1, SL1], F32, tag="smr")
                nc.scalar.sqrt(q2[:], q1[:])
                for k in range(SL1 // P):
                    pq = psm.tile([P, 1], F32, tag="mc", bufs=1)
                    nc.tensor.matmul(pq[:], q2[0:1, k * P : (k + 1) * P],
                                     ones1[:], start=True, stop=True)
                    idx = sl * (SL1 // P) + k
                    nc.vector.reciprocal(rq[:, idx : idx + 1], pq[:])

            # ---------------- phases 2+3, pipelined over j-panels --------
            def issue_ph2(jp):
                jps = slice(jp * JP, (jp + 1) * JP)
                ktb = ktp.tile([P, BT, JP], FP8, tag="ktb")
                nc.sync.dma_start(ktb[:], kt_r[:, :, jps])
                psb = ps.tile([P, JP], F32, tag="ps")
                nc.tensor.matmul(psb[:], ones_rowb[:], rk_all[0:1, jps],
                                 start=True, stop=True)
                rkb = rkp.tile([P, JP], F32, tag="rkb")
                nc.scalar.copy(rkb[:], psb[:])
                csum = pscs.tile([1, JP], F32, tag="cs")
                E = ep.tile([P, CT, JP], BF16, tag="e")
                for ip in range(CT):
                    isl = slice(ip * P, (ip + 1) * P)
                    pss = ps.tile([P, JP], F32, tag="ps")
                    for tb in range(BT // 2):
                        nc.tensor.matmul(
                            pss[:], qt8[:, 2 * tb : 2 * tb + 2, isl],
                            ktb[:, 2 * tb : 2 * tb + 2, :],
                            start=(tb == 0), stop=(tb == BT // 2 - 1),
                            perf_mode=DR,
                        )
                    tm = tmppool.tile([P, JP], F32, tag="tmp")
                    nc.vector.tensor_tensor(tm[:], pss[:], rkb[:], OP.mult)
                    nc.scalar.activation(E[:, ip, :], tm[:], AF.Exp,
                                         scale=rq[:, ip : ip + 1])
                    nc.tensor.matmul(csum[:], ones_colb[:], E[:, ip, :],
                                     start=(ip == 0), stop=(ip == CT - 1))
                csrow = csrpool.tile([1, JP], F32, tag="csr")
                nc.scalar.copy(csrow[:], csum[:])
                rc = rcpool.tile([P, JP // P], F32, tag="rc")
                for k in range(JP // P):
                    tcol = psm.tile([P, 1], F32, tag="m")
                    nc.tensor.matmul(tcol[:],
                                     csrow[0:1, k * P : (k + 1) * P],
                                     ones1[:], start=True, stop=True)
                    nc.vector.reciprocal(rc[:, k : k + 1], tcol[:])
                return E, rc

            def issue_ph3(jp, E, rc):
                for k in range(JP // P):
                    jrow = jp * JP + k * P
                    for bsl in range(NB3):
                        bs = slice(bsl * BS3, (bsl + 1) * BS3)
                        psz = ps.tile([P, BS3], F32, tag="ps")
                        for ic in range(CT):
                            nc.tensor.matmul(
                                psz[:], E[:, ic, k * P : (k + 1) * P],
                                xbft[:, ic, bs],
                                start=(ic == 0), stop=(ic == CT - 1),
                            )
                        zt = zpool.tile([P, BS3], F32, tag="z")
                        nc.scalar.mul(zt[:], psz[:], rc[:, k : k + 1])
                        nc.sync.dma_start(z_d[jrow : jrow + P, bs], zt[:])

            prev = None
            for jp in range(NJP):
                cur = issue_ph2(jp)
                if prev is not None:
                    issue_ph3(jp - 1, *prev)
                prev = cur
            issue_ph3(NJP - 1, *prev)

    nc.compile()
    return nc


def _host_prep(Wk, Wq, Wk0, Wq0, C):
    def wpack(W, W0):
        w = np.zeros((C + 2 * P, C), np.float32)
        w[:C] = W.T * WSCALE
        w[C] = W0.reshape(C) * WSCALE
        return w.astype(FP8NP)

    return wpack(Wk, Wk0), wpack(Wq, Wq0)


_CACHE = {}


def kernel(X, Wk, Wq, Wk0, Wq0):
    global LAST_EXEC_NS
    X = np.asarray(X, dtype=np.float32)
    Wk = np.asarray(Wk, dtype=np.float32)
    Wq = np.asarray(Wq, dtype=np.float32)
    Wk0 = np.asarray(Wk0, dtype=np.float32)
    Wq0 = np.asarray(Wq0, dtype=np.float32)
    N, C, B = X.shape
    assert N == N_CORES

    from concourse.bass_utils import run_bass_kernel_spmd

    key = (C, B)
    if key not in _CACHE:
        _CACHE[key] = build_program(C, B)
    nc = _CACHE[key]

    wk8, wq8 = _host_prep(Wk, Wq, Wk0, Wq0, C)
    in_maps = []
    for n in range(N):
        x8 = np.zeros((C + 2 * P, B), np.float32)
        x8[:C] = X[n]
        x8[C] = 1.0
        in_maps.append({
            "x8": x8.astype(FP8NP),
            "xbf": X[n].astype(BF16NP),
            "wk8": wk8,
            "wq8": wq8,
        })

    trace = bool(os.environ.get("BASS_KERNEL_TRACE"))
    kw = {}
    if trace:
        kw["trace"] = True
        td = os.environ.get("BASS_KERNEL_TMPDIR")
        if td:
            os.makedirs(td, exist_ok=True)
            kw["tmpdir"] = td
    t0 = time.time()
    res = run_bass_kernel_spmd(nc, in_maps, core_ids=list(range(N_CORES)),
                               **kw)
    LAST_EXEC_NS = int((time.time() - t0) * 1e9)
    if getattr(res, "exec_time_ns", None):
        LAST_EXEC_NS = int(res.exec_time_ns)
    out = np.stack([res.results[n]["z"] for n in range(N)], axis=0)
    return out.astype(np.float32)


if __name__ == "__main__":
    # small-scale self-test vs numpy
    C, B = 512, 512
    rng = np.random.default_rng(1)
    Xs = rng.standard_normal((N_CORES, C, B), dtype=np.float32)
    bound = float(np.sqrt(6.0 / (C + C)))
    Wks = rng.uniform(-bound, bound, (C, C)).astype(np.float32)
    Wqs = rng.uniform(-bound, bound, (C, C)).astype(np.float32)
    Wk0s = rng.standard_normal((C, 1)).astype(np.float32) * 0.01
    Wq0s = rng.standard_normal((C, 1)).astype(np.float32) * 0.01

    def ref(X, Wk, Wq, Wk0, Wq0):
        K = np.einsum("ij,njb->nib", Wk, X) + Wk0
        Q = np.einsum("ij,njb->nib", Wq, X) + Wq0
        DK2 = np.sum(K * K, axis=2)
        DQ2 = np.sum(Q * Q, axis=2)
        DQK = np.sqrt(np.maximum(DQ2[:, :, None] * DK2[:, None, :], 1e-12))
        Y = np.einsum("nib,njb->nij", Q, K) / DQK
        Y = Y - Y.max(axis=1, keepdims=True)
        E = np.exp(Y)
        SM = E / E.sum(axis=1, keepdims=True)
        return np.einsum("ncb,ncj->njb", X, SM)

    expected = ref(
        Xs.astype(np.float64), Wks.astype(np.float64),
        Wqs.astype(np.float64), Wk0s.astype(np.float64),
        Wq0s.astype(np.float64),
    )
    actual = kernel(Xs, Wks, Wqs, Wk0s, Wq0s)
    rel = np.linalg.norm(actual - expected) / np.linalg.norm(expected)
    print(f"small test relative error: {rel:.3e}")
    print(f"wall ns: {LAST_EXEC_NS}")
